# revision 1
# baseline (speedup 1.0000x reference)
"""CenterLoss (segment_reduce) Trainium2 kernel.

Math (faithful to the reference):
  preds = argmax_c logits[n, c, h, w]          (softmax is monotone -> skip it)
  s1[p] = sum_c x, s2[p] = sum_c x^2 per pixel p=(n,h,w)
  per (n, cls): cnt = #pixels with preds==cls, S1 = sum s1, S2 = sum s2
  K = max(cnt,1)*C; sq_dev = max(S2 - S1^2/K, 0)
  loss = sum_cls mean_n( cnt>0 ? sqrt(sq_dev) : 0 )

Device strategy (8 cores, data-parallel over 16 units = (n, H-slab of 128)):
  Each core takes 2 units of shape (C=19, 128, 1024) fp32.  SBUF layout puts
  H on partitions and (C, W) on the free dim, so per-pixel class reductions
  are free-dim ops at full 128-partition occupancy:
    m  = max over c   : pairwise TT tree, fp32 (exactness of the argmax mask)
    s1 = sum over c   : pairwise TT tree in bf16 (from an ACT bf16 cast)
    s2 = sum over c x^2: pairwise TT tree in bf16 (from ACT Square, bf16)
    per class c: STT (x_c ==) m   -> eq mask + fused count accum
                 STT eq * s1      -> fused S1 accum
                 STT eq * s2      -> fused S2 accum
  Contiguous trees avoid the ~1.6 cyc/elem strided-read penalty of
  tensor_reduce with a strided innermost dim; bf16 tree levels ride the DVE
  2x_1P tensor_tensor mode.  The STT passes are fp32 (the fused
  scalar_tensor_tensor opcode has no fast mode, and bf16 outputs measured
  slower).  Per-(partition, class) partial sums are DMA'd out; host sums the
  tiny partials and applies the final formula.  `target` is unused by the
  reference and never shipped.

  Measured on trn2 via axon: ~252 us HW exec, with the DVE stream fully
  packed (zero >300 ns gaps): ~22 us head (fixed startup + first chunk
  load) + ~226 us DVE + ~10 us tail drain.
"""

import numpy as np


def _ensure_ntff_hook():
    """bass_utils' trace path imports antenv.axon_hooks, which this image
    lacks.  Install a shim backed by trn_agent_boot's ctypes hook so a
    BASS_TRACE=1 environment doesn't crash the run (and tracing works)."""
    import sys
    import types

    try:
        import antenv.axon_hooks  # noqa: F401
        return
    except ImportError:
        pass
    try:
        from trn_agent_boot.trn_boot import _ntff_profile_via_ctypes

        hook = _ntff_profile_via_ctypes("/opt/axon/libaxon_pjrt.so")
    except Exception:
        hook = None
    mod = types.ModuleType("antenv.axon_hooks")
    mod.get_axon_ntff_profile_hook = lambda: hook
    mod.set_axon_ntff_profile_hook = lambda h: None
    sys.modules["antenv.axon_hooks"] = mod

N, C, H, W = 4, 19, 512, 1024
NCORES = 8
SLABS = 4                 # H split into 4 slabs of 128 partitions
P = H // SLABS            # 128
UNITS = [(n, s) for n in range(N) for s in range(SLABS)]   # 16 units
UPC = len(UNITS) // NCORES                                  # 2 units per core
WCHUNK = 512
NCHUNKS = W // WCHUNK

_CACHE = {}

# Per-core chunk schedule: (unit, wchunk-slot, lo, wid).
CHUNKS = [(u, ch, 0, WCHUNK) for u in range(UPC) for ch in range(NCHUNKS)]
SLOT_UNIT = [c[0] for c in CHUNKS]
NSLOTS = len(CHUNKS)


def _build_nc():
    from contextlib import ExitStack

    import concourse.tile as tile
    from concourse import bacc, mybir

    f32 = mybir.dt.float32
    bf16 = mybir.dt.bfloat16
    Alu = mybir.AluOpType
    Act = mybir.ActivationFunctionType

    nc = bacc.Bacc("TRN2", target_bir_lowering=False, debug=False)
    # Host pre-arranges each core's shard as (unit, wchunk, h, c, w) so one
    # chunk load is 128 fully contiguous 38.9 KB runs (descriptor-overhead-
    # bound 2 KB runs cost ~22.8 us/chunk; contiguous is ~13 us).
    x_d = nc.dram_tensor(
        "x", [UPC, NCHUNKS, P, C, WCHUNK], f32, kind="ExternalInput"
    ).ap()
    out_d = nc.dram_tensor(
        "stats", [NSLOTS, P, 3 * C], f32, kind="ExternalOutput"
    ).ap()

    with tile.TileContext(nc) as tc, ExitStack() as ctx:
        xpool = ctx.enter_context(tc.tile_pool(name="x", bufs=2))
        bfpool = ctx.enter_context(tc.tile_pool(name="bf", bufs=1))
        tpool = ctx.enter_context(tc.tile_pool(name="tree", bufs=1))
        eqpool = ctx.enter_context(tc.tile_pool(name="eq", bufs=4))
        jpool = ctx.enter_context(tc.tile_pool(name="junk", bufs=4))
        cpool = ctx.enter_context(tc.tile_pool(name="cols", bufs=2))

        def tree(src, wid, op, dt, out_dt, tag):
            """Pairwise-reduce the C=19 rows of 3-dim AP `src` (P, 19, wid)
            along the row dim via contiguous tensor_tensor ops, no copies:
            leftovers (src row 18, level-1 row 8) are folded in at the end.
            Intermediate levels use dtype dt; the final level writes a
            (P, wid) result of out_dt.  Returns that AP."""
            assert C == 19
            t = tpool.tile([P, 10, wid], dt, tag=tag, name=f"tree_{tag}")
            if dt == out_dt:
                res = t[:, 9, :]
            else:
                res = tpool.tile(
                    [P, wid], out_dt, tag=tag + "o", name=f"tree_{tag}o"
                )[:]
            tt = nc.vector.tensor_tensor
            tt(out=t[:, 0:9, :], in0=src[:, 0:9, :], in1=src[:, 9:18, :], op=op)
            tt(out=t[:, 0:4, :], in0=t[:, 0:4, :], in1=t[:, 4:8, :], op=op)
            tt(out=t[:, 0:2, :], in0=t[:, 0:2, :], in1=t[:, 2:4, :], op=op)
            tt(out=t[:, 0, :], in0=t[:, 0, :], in1=t[:, 1, :], op=op)
            tt(out=t[:, 0, :], in0=t[:, 0, :], in1=t[:, 8, :], op=op)
            tt(out=res, in0=t[:, 0, :], in1=src[:, 18, :], op=op)
            return res

        for slot, (u, ch, lo, wid) in enumerate(CHUNKS):
            xt = xpool.tile([P, C, wid], f32, tag="x", name=f"x{slot}")
            nc.sync.dma_start(xt[:], x_d[u, ch, :, :, lo:lo + wid])

            # bf16 casts on ScalarE (otherwise idle).  Square is issued
            # first and its tree runs before s1's, so at kernel start each
            # tree's input is ready when the m-tree finishes (no DVE stall
            # on the first chunk's ACT latency).
            sq = bfpool.tile([P, C, wid], bf16, tag="sq", name=f"sq{slot}")
            nc.scalar.activation(sq[:], xt[:], Act.Square)
            xb = bfpool.tile([P, C, wid], bf16, tag="xb", name=f"xb{slot}")
            nc.scalar.activation(xb[:], xt[:], Act.Identity)

            m = tree(xt[:], wid, Alu.max, f32, f32, "m")
            s2 = tree(sq[:], wid, Alu.add, bf16, f32, "s2")
            s1 = tree(xb[:], wid, Alu.add, bf16, f32, "s1")

            cols = cpool.tile([P, 3 * C], f32, tag="cols", name=f"cols{slot}")
            for c in range(C):
                eq = eqpool.tile([P, wid], f32, tag="eq", name=f"eq{slot}_{c}")
                nc.vector.scalar_tensor_tensor(
                    out=eq[:], in0=xt[:, c, :], scalar=1.0, in1=m,
                    op0=Alu.mult, op1=Alu.is_equal,
                    accum_out=cols[:, c:c + 1],
                )
                j1 = jpool.tile([P, wid], f32, tag="junk", name=f"j1_{slot}_{c}")
                nc.vector.scalar_tensor_tensor(
                    out=j1[:], in0=eq[:], scalar=1.0, in1=s1,
                    op0=Alu.mult, op1=Alu.mult,
                    accum_out=cols[:, C + c:C + c + 1],
                )
                j2 = jpool.tile([P, wid], f32, tag="junk", name=f"j2_{slot}_{c}")
                nc.vector.scalar_tensor_tensor(
                    out=j2[:], in0=eq[:], scalar=1.0, in1=s2,
                    op0=Alu.mult, op1=Alu.mult,
                    accum_out=cols[:, 2 * C + c:2 * C + c + 1],
                )

            nc.sync.dma_start(out_d[slot], cols[:])

    nc.compile()
    return nc


def _get_nc():
    if "nc" not in _CACHE:
        _CACHE["nc"] = _build_nc()
    return _CACHE["nc"]


def _make_shards(logits):
    shards = []
    for k in range(NCORES):
        units = [UNITS[UPC * k + i] for i in range(UPC)]
        arr = np.stack(
            [logits[n, :, s * P:(s + 1) * P, :] for (n, s) in units]
        ).astype(np.float32, copy=False)            # (UPC, C, P, W)
        arr = arr.reshape(UPC, C, P, NCHUNKS, WCHUNK)
        arr = arr.transpose(0, 3, 2, 1, 4)           # (UPC, NCH, P, C, WC)
        shards.append(np.ascontiguousarray(arr))
    return shards


def _finish(results):
    per_n = np.zeros((N, 3, C), dtype=np.float64)
    for k in range(NCORES):
        arr = np.asarray(results[k]["stats"], dtype=np.float64)
        a = arr.reshape(NSLOTS, P, 3, C).sum(axis=1)   # (NSLOTS, 3, C)
        for slot in range(NSLOTS):
            n, _s = UNITS[UPC * k + SLOT_UNIT[slot]]
            per_n[n] += a[slot]
    cnt, S1, S2 = per_n[:, 0], per_n[:, 1], per_n[:, 2]
    K = np.maximum(cnt, 1.0) * C
    sq_dev = np.maximum(S2 - S1 * S1 / K, 0.0)
    norms = np.where(cnt > 0, np.sqrt(sq_dev), 0.0)
    loss = norms.mean(axis=0).sum()
    return np.array(loss, dtype=np.float32)


def kernel(**inputs):
    _ensure_ntff_hook()
    from concourse.bass_utils import run_bass_kernel_spmd

    logits = np.asarray(inputs["logits"])
    assert logits.shape == (N, C, H, W), logits.shape
    nc = _get_nc()
    shards = _make_shards(logits)
    in_maps = [{"x": shards[k]} for k in range(NCORES)]
    res = run_bass_kernel_spmd(nc, in_maps, list(range(NCORES)))
    return _finish(res.results)



# revision 3
# speedup vs baseline: 1.9504x; 1.9504x over previous
"""CenterLoss (segment_reduce) Trainium2 kernel — PE segment-sum version.

Math (faithful to the reference):
  preds = argmax_c logits[n, c, h, w]          (softmax is monotone -> skip it)
  per (n, cls): cnt = #pixels with preds==cls, S1 = sum_{pix,ch} x,
                S2 = sum_{pix,ch} x^2 over pixels of that class
  K = max(cnt,1)*C; sq_dev = max(S2 - S1^2/K, 0)
  loss = sum_cls mean_n( cnt>0 ? sqrt(sq_dev) : 0 )

Device strategy (8 cores, data-parallel; core k owns sample n=k//2's
half of H):  pixels live on (128 partitions x 2048 w-slots) per core, in
4 chunks of 512 slots.  Input is cast to fp16 on the host (halves DMA;
measured rel err 4.7e-4 incl. fp16 argmax ties double-matching ~0.09%
of pixels — well inside the 2e-2 gate).

Per chunk, the host ships a (P, 20, 512) fp16 tile: rows 0:19 are the
19 class channels, row 19 is ones.  On device:
  ScalarE: rows 20:39 = Square(rows 0:19)       (one ACT op)
  DVE:     m = pairwise fp16 max-tree over the 19 rows (2x_1P mode)
           eq = is_equal(rows 0:19, broadcast m) -> one-hot (P,19,512)
  PE:      per w-slot: psum[32*(w%4)+c, j] += eq[:,c,w] * mov[:,j,w]
           i.e. matmul(lhsT=eq[:,:,w] (128x19), rhs=mov[:,0:39,w]
           (128x39)) accumulated over all 2048 slots into one PSUM
           tile; the w%4 col-grouping gives 4 concurrent 32-col PE
           tiles so small matmuls overlap.
psum cols 0:19 are per-(cls, ch) S1 breakdown, col 19 is cnt, cols
20:39 the S2 breakdown.  One tiny ACT copy + DMA ships (128, 39) f32
per core; the host folds the 4 col-groups, sums the channel breakdowns
and applies the final sqrt/mean formula.  `target` is unused by the
reference and never shipped.
"""

import numpy as np


def _ensure_ntff_hook():
    """bass_utils' trace path imports antenv.axon_hooks, which this image
    lacks.  Install a shim backed by trn_agent_boot's ctypes hook so a
    BASS_TRACE=1 environment doesn't crash the run (and tracing works)."""
    import sys
    import types

    try:
        import antenv.axon_hooks  # noqa: F401
        return
    except ImportError:
        pass
    try:
        from trn_agent_boot.trn_boot import _ntff_profile_via_ctypes

        hook = _ntff_profile_via_ctypes("/opt/axon/libaxon_pjrt.so")
    except Exception:
        hook = None
    mod = types.ModuleType("antenv.axon_hooks")
    mod.get_axon_ntff_profile_hook = lambda: hook
    mod.set_axon_ntff_profile_hook = lambda h: None
    sys.modules["antenv.axon_hooks"] = mod

N, C, H, W = 4, 19, 512, 1024
NCORES = 8
SLABS = 4                 # H split into 4 slabs of 128 partitions
P = H // SLABS            # 128
UNITS = [(n, s) for n in range(N) for s in range(SLABS)]   # 16 units
UPC = len(UNITS) // NCORES                                  # 2 units per core
WCHUNK = 512
NCHUNKS = UPC * (W // WCHUNK)  # 4 chunks of 512 w-slots per core
NGRP = 4                       # PE col-groups (PSUM partition offsets 32j)
MROWS = 40                     # mov tile rows: 0:19 x, 19 ones, 20:39 sq

_CACHE = {}


def _build_nc():
    from contextlib import ExitStack

    import concourse.tile as tile
    from concourse import bacc, mybir

    f32 = mybir.dt.float32
    f16 = mybir.dt.float16
    Alu = mybir.AluOpType
    Act = mybir.ActivationFunctionType

    nc = bacc.Bacc("TRN2", target_bir_lowering=False, debug=False)
    # Host pre-arranges each core's shard as (chunk, h, row, w) fp16 with
    # rows 0:19 = channels and row 19 = 1.0, so one chunk load is 128
    # contiguous 20 KB runs.
    x_d = nc.dram_tensor(
        "x", [NCHUNKS, P, 20, WCHUNK], f16, kind="ExternalInput"
    ).ap()
    out_d = nc.dram_tensor("stats", [P, 2 * C + 1], f32, kind="ExternalOutput").ap()

    with tile.TileContext(nc) as tc, ExitStack() as ctx:
        movpool = ctx.enter_context(tc.tile_pool(name="mov", bufs=3))
        eqpool = ctx.enter_context(tc.tile_pool(name="eq", bufs=2))
        tpool = ctx.enter_context(tc.tile_pool(name="tree", bufs=2))
        spool = ctx.enter_context(tc.tile_pool(name="stats", bufs=1))
        ppool = ctx.enter_context(tc.tile_pool(name="ps", bufs=1, space="PSUM"))

        ps = ppool.tile([P, 2 * C + 1], f32, name="psacc")

        for ch in range(NCHUNKS):
            mov = movpool.tile([P, MROWS, WCHUNK], f16, tag="mov", name=f"mov{ch}")
            nc.sync.dma_start(mov[:, 0:20, :], x_d[ch])

            # squares on the otherwise-idle ScalarE
            nc.scalar.activation(mov[:, 20:39, :], mov[:, 0:19, :], Act.Square)

            # fp16 pairwise max-tree over the 19 channel rows (DVE 2x_1P)
            t = tpool.tile([P, 9, WCHUNK], f16, tag="t", name=f"t{ch}")
            m = tpool.tile([P, WCHUNK], f16, tag="m", name=f"m{ch}")
            tt = nc.vector.tensor_tensor
            tt(out=t[:, 0:9, :], in0=mov[:, 0:9, :], in1=mov[:, 9:18, :], op=Alu.max)
            tt(out=t[:, 0:4, :], in0=t[:, 0:4, :], in1=t[:, 4:8, :], op=Alu.max)
            tt(out=t[:, 0:2, :], in0=t[:, 0:2, :], in1=t[:, 2:4, :], op=Alu.max)
            tt(out=t[:, 0, :], in0=t[:, 0, :], in1=t[:, 1, :], op=Alu.max)
            tt(out=t[:, 0, :], in0=t[:, 0, :], in1=t[:, 8, :], op=Alu.max)
            tt(out=m[:], in0=t[:, 0, :], in1=mov[:, 18, :], op=Alu.max)

            # one-hot masks: eq[p, c, w] = (x[p, c, w] == m[p, w])
            eq = eqpool.tile([P, C, WCHUNK], f16, tag="eq", name=f"eq{ch}")
            mb = m[:].unsqueeze(1).broadcast_to([P, C, WCHUNK])
            tt(out=eq[:], in0=mov[:, 0:19, :], in1=mb, op=Alu.is_equal)

            # segment sums on the PE: one small matmul per w-slot,
            # accumulated into 4 concurrent PSUM col-groups
            for w in range(WCHUNK):
                g = 32 * (w % NGRP)
                nc.tensor.matmul(
                    out=ps[g:g + C, :],
                    lhsT=eq[:, :, w],
                    rhs=mov[:, 0:39, w],
                    start=(ch == 0 and w < NGRP),
                    stop=(ch == NCHUNKS - 1 and w >= WCHUNK - NGRP),
                    tile_position=(0, g),
                )

        stats = spool.tile([P, 2 * C + 1], f32, name="stats")
        nc.scalar.activation(stats[:], ps[:], Act.Copy)
        nc.sync.dma_start(out_d[:], stats[:])

    nc.compile()
    return nc


def _get_nc():
    if "nc" not in _CACHE:
        _CACHE["nc"] = _build_nc()
    return _CACHE["nc"]


def _make_shards(logits):
    logits = np.asarray(logits).astype(np.float16)
    shards = []
    for k in range(NCORES):
        arr = np.ones((NCHUNKS, P, 20, WCHUNK), dtype=np.float16)
        for ch in range(NCHUNKS):
            n, s = UNITS[UPC * k + ch // 2]
            b = ch % 2
            blk = logits[n, :, s * P:(s + 1) * P, b * WCHUNK:(b + 1) * WCHUNK]
            arr[ch, :, 0:19, :] = blk.transpose(1, 0, 2)
        shards.append(arr)
    return shards


def _finish(results):
    per_n = np.zeros((N, C, 2 * C + 1), dtype=np.float64)
    for k in range(NCORES):
        st = np.asarray(results[k]["stats"], dtype=np.float64)  # (128, 39)
        acc = np.zeros((C, 2 * C + 1))
        for j in range(NGRP):
            acc += st[32 * j:32 * j + C, :]
        per_n[k // UPC] += acc
    S1 = per_n[:, :, 0:19].sum(axis=2)
    cnt = per_n[:, :, 19]
    S2 = per_n[:, :, 20:39].sum(axis=2)
    K = np.maximum(cnt, 1.0) * C
    sq_dev = np.maximum(S2 - S1 * S1 / K, 0.0)
    norms = np.where(cnt > 0, np.sqrt(sq_dev), 0.0)
    loss = norms.mean(axis=0).sum()
    return np.array(loss, dtype=np.float32)


def kernel(**inputs):
    _ensure_ntff_hook()
    from concourse.bass_utils import run_bass_kernel_spmd

    logits = np.asarray(inputs["logits"])
    assert logits.shape == (N, C, H, W), logits.shape
    nc = _get_nc()
    shards = _make_shards(logits)
    in_maps = [{"x": shards[k]} for k in range(NCORES)]
    res = run_bass_kernel_spmd(nc, in_maps, list(range(NCORES)))
    return _finish(res.results)


# revision 5
# speedup vs baseline: 2.2991x; 1.1788x over previous
"""CenterLoss (segment_reduce) Trainium2 kernel — PE segment-sum version.

Math (faithful to the reference):
  preds = argmax_c logits[n, c, h, w]          (softmax is monotone -> skip it)
  per (n, cls): cnt = #pixels with preds==cls, S1 = sum_{pix,ch} x,
                S2 = sum_{pix,ch} x^2 over pixels of that class
  K = max(cnt,1)*C; sq_dev = max(S2 - S1^2/K, 0)
  loss = sum_cls mean_n( cnt>0 ? sqrt(sq_dev) : 0 )

Device strategy (8 cores, data-parallel; core k owns sample n=k//2's
half of H): pixels live on (128 partitions x 2048 w-slots) per core, in
8 segments of 256 slots.  Input is cast to fp16 on the host (halves
DMA; measured rel err 4.7e-4 incl. fp16 argmax ties double-matching
~0.09% of pixels — well inside the 2e-2 gate).

Per segment the host ships a (P, 20, 64, 4) fp16 tile: rows 0:19 the
19 class channels, row 19 ones, w-slots viewed as (wb, k) blocks of 4.
On device:
  ScalarE: rows 20:39 = Square(rows 0:19)       (one ACT op)
  DVE:     m = pairwise fp16 max-tree over the 19 rows (2x_1P mode)
           eq = is_equal(x, broadcast m) -> one-hot, written through a
           transposed AP into an interleaved (P, wb, c, k) tile so each
           slot's 19-col stationary sits at a dense 8-byte stride (10
           SBUF lines per LDWEIGHTS instead of 19) while the DVE write
           keeps a step-1 innermost dim (stays in 2x mode).
  PE:      per slot: psum[32k + c, j] += eq[:, c] * mov[:, j] — a
           (128x19)x(128x39) matmul accumulated over all 2048 slots,
           rotating k=0..3 across four 32-col PSUM groups so the small
           matmuls overlap in the array.
psum cols 0:19 are the per-(cls, ch) S1 breakdown, col 19 cnt, cols
20:39 the S2 breakdown.  One ACT copy + tiny DMA ships (128, 39) f32
per core; the host folds the 4 col-groups, sums the channel breakdowns
and applies the final sqrt/mean formula.  `target` is unused by the
reference and never shipped.
"""

import numpy as np


def _ensure_ntff_hook():
    """bass_utils' trace path imports antenv.axon_hooks, which this image
    lacks.  Install a shim backed by trn_agent_boot's ctypes hook so a
    BASS_TRACE=1 environment doesn't crash the run (and tracing works)."""
    import sys
    import types

    try:
        import antenv.axon_hooks  # noqa: F401
        return
    except ImportError:
        pass
    try:
        from trn_agent_boot.trn_boot import _ntff_profile_via_ctypes

        hook = _ntff_profile_via_ctypes("/opt/axon/libaxon_pjrt.so")
    except Exception:
        hook = None
    mod = types.ModuleType("antenv.axon_hooks")
    mod.get_axon_ntff_profile_hook = lambda: hook
    mod.set_axon_ntff_profile_hook = lambda h: None
    sys.modules["antenv.axon_hooks"] = mod

N, C, H, W = 4, 19, 512, 1024
NCORES = 8
SLABS = 4                 # H split into 4 slabs of 128 partitions
P = H // SLABS            # 128
UNITS = [(n, s) for n in range(N) for s in range(SLABS)]   # 16 units
UPC = len(UNITS) // NCORES                                  # 2 units per core
SEGW = 256
NSEG = (UPC * W) // SEGW   # 8 segments of 256 w-slots per core
NGRP = 4                   # PE col-groups (PSUM partition offsets 32k)
WB = SEGW // NGRP          # 64 slot-blocks per segment
MROWS = 40                 # mov tile rows: 0:19 x, 19 ones, 20:39 sq

_CACHE = {}


def _build_nc():
    from contextlib import ExitStack

    import concourse.tile as tile
    from concourse import bacc, mybir

    f32 = mybir.dt.float32
    f16 = mybir.dt.float16
    Alu = mybir.AluOpType
    Act = mybir.ActivationFunctionType

    nc = bacc.Bacc("TRN2", target_bir_lowering=False, debug=False)
    # Host pre-arranges each core's shard as (seg, h, row, wb, k) fp16 with
    # rows 0:19 = channels and row 19 = 1.0, so one segment load is 128
    # contiguous 10 KB runs.
    x_d = nc.dram_tensor(
        "x", [NSEG, P, 20, WB, NGRP], f16, kind="ExternalInput"
    ).ap()
    out_d = nc.dram_tensor("stats", [P, 2 * C + 1], f32, kind="ExternalOutput").ap()

    with tile.TileContext(nc) as tc, ExitStack() as ctx:
        movpool = ctx.enter_context(tc.tile_pool(name="mov", bufs=4))
        eqpool = ctx.enter_context(tc.tile_pool(name="eq", bufs=3))
        tpool = ctx.enter_context(tc.tile_pool(name="tree", bufs=2))
        spool = ctx.enter_context(tc.tile_pool(name="stats", bufs=1))
        ppool = ctx.enter_context(tc.tile_pool(name="ps", bufs=1, space="PSUM"))

        ps = ppool.tile([P, 2 * C + 1], f32, name="psacc")

        for s in range(NSEG):
            mov = movpool.tile([P, MROWS, WB, NGRP], f16, tag="mov", name=f"mov{s}")
            nc.sync.dma_start(mov[:, 0:20, :, :], x_d[s])

            # squares on the otherwise-idle ScalarE
            nc.scalar.activation(
                mov[:, 20:39, :, :], mov[:, 0:19, :, :], Act.Square
            )

            # fp16 pairwise max-tree over the 19 channel rows (DVE 2x_1P)
            t = tpool.tile([P, 9, WB, NGRP], f16, tag="t", name=f"t{s}")
            m = tpool.tile([P, WB, NGRP], f16, tag="m", name=f"m{s}")
            tt = nc.vector.tensor_tensor
            tt(out=t[:, 0:9], in0=mov[:, 0:9], in1=mov[:, 9:18], op=Alu.max)
            tt(out=t[:, 0:4], in0=t[:, 0:4], in1=t[:, 4:8], op=Alu.max)
            tt(out=t[:, 0:2], in0=t[:, 0:2], in1=t[:, 2:4], op=Alu.max)
            tt(out=t[:, 0], in0=t[:, 0], in1=t[:, 1], op=Alu.max)
            tt(out=t[:, 0], in0=t[:, 0], in1=t[:, 8], op=Alu.max)
            tt(out=m[:], in0=t[:, 0], in1=mov[:, 18], op=Alu.max)

            # one-hot masks: eq[p, wb, c, k] = (x[p, c, wb, k] == m[p, wb, k])
            # (interleaved tile; DVE writes through a transposed AP, so the
            # innermost iterated dim keeps step 1 and 2x mode)
            eq = eqpool.tile([P, WB, C, NGRP], f16, tag="eq", name=f"eq{s}")
            eqv = eq[:].transpose([0, 2, 1, 3])          # (P, C, WB, NGRP)
            mb = m[:].unsqueeze(1).broadcast_to([P, C, WB, NGRP])
            tt(out=eqv, in0=mov[:, 0:19], in1=mb, op=Alu.is_equal)

            # segment sums on the PE: one small matmul per w-slot, rotating
            # k across 4 concurrent PSUM col-groups
            for wb in range(WB):
                for k in range(NGRP):
                    nc.tensor.matmul(
                        out=ps[32 * k:32 * k + C, :],
                        lhsT=eq[:, wb, :, k],
                        rhs=mov[:, 0:39, wb, k],
                        start=(s == 0 and wb == 0),
                        stop=(s == NSEG - 1 and wb == WB - 1),
                        tile_position=(0, 32 * k),
                    )

        stats = spool.tile([P, 2 * C + 1], f32, name="stats")
        nc.scalar.activation(stats[:], ps[:], Act.Copy)
        nc.sync.dma_start(out_d[:], stats[:])

    nc.compile()
    return nc


def _get_nc():
    if "nc" not in _CACHE:
        _CACHE["nc"] = _build_nc()
    return _CACHE["nc"]


def _make_shards(logits):
    logits = np.asarray(logits).astype(np.float16)
    shards = []
    for k in range(NCORES):
        arr = np.ones((NSEG, P, 20, SEGW), dtype=np.float16)
        segs_per_unit = W // SEGW
        for s in range(NSEG):
            n, sl = UNITS[UPC * k + s // segs_per_unit]
            b = s % segs_per_unit
            blk = logits[n, :, sl * P:(sl + 1) * P, b * SEGW:(b + 1) * SEGW]
            arr[s, :, 0:19, :] = blk.transpose(1, 0, 2)
        shards.append(arr.reshape(NSEG, P, 20, WB, NGRP))
    return shards


def _finish(results):
    per_n = np.zeros((N, C, 2 * C + 1), dtype=np.float64)
    for k in range(NCORES):
        st = np.asarray(results[k]["stats"], dtype=np.float64)  # (128, 39)
        acc = np.zeros((C, 2 * C + 1))
        for g in range(NGRP):
            acc += st[32 * g:32 * g + C, :]
        per_n[k // UPC] += acc
    S1 = per_n[:, :, 0:19].sum(axis=2)
    cnt = per_n[:, :, 19]
    S2 = per_n[:, :, 20:39].sum(axis=2)
    K = np.maximum(cnt, 1.0) * C
    sq_dev = np.maximum(S2 - S1 * S1 / K, 0.0)
    norms = np.where(cnt > 0, np.sqrt(sq_dev), 0.0)
    loss = norms.mean(axis=0).sum()
    return np.array(loss, dtype=np.float32)


def kernel(**inputs):
    _ensure_ntff_hook()
    from concourse.bass_utils import run_bass_kernel_spmd

    logits = np.asarray(inputs["logits"])
    assert logits.shape == (N, C, H, W), logits.shape
    nc = _get_nc()
    shards = _make_shards(logits)
    in_maps = [{"x": shards[k]} for k in range(NCORES)]
    res = run_bass_kernel_spmd(nc, in_maps, list(range(NCORES)))
    return _finish(res.results)
